# revision 8
# baseline (speedup 1.0000x reference)
"""Multi-head attention (B=2, N=4096, C=512, H=8) on 8 TRN2 NeuronCores.

Sharding: core c handles batch b = c//4 and heads {2*(c%4), 2*(c%4)+1}
(data parallel over B, tensor parallel over heads).  Each core computes its
two heads' full attention plus their slice of the output projection; the
per-core [C, N] projection partials are summed per batch on the host
(the "proj all-reduce") and the projection bias is added there too.

Device-side math (all matmuls bf16 inputs, fp32 PSUM accumulation):
  qT/kT/vT = W_blk @ x^T + b          [128(2 heads x 64), 4096]
  S^T(jt, ic) = K_jt @ Q_ic^T         scores transposed, both heads row-packed
  P^T = exp(SCALE * S^T)              ScalarE, no max subtraction (|s|<=2.7)
  O_raw^T = [V; ones]^T-style:  acc[0:65] = [v | 1]^T-matmul over j tiles
            (row 64 of acc is the softmax denominator)
  O^T = O_raw^T * (1/denom)           normalize after the V matmul
  out^T partial = Wp_blk^T @ O^T      [512, 4096] fp32 -> DRAM
"""

import numpy as np
import ml_dtypes

B, N, C = 2, 4096, 512
H, Dh = 8, 64
SCALE = Dh**-0.5
NCORES = 8
HPC = 2  # heads per core
ICW = 512  # i-chunk width
NIC = N // ICW  # 8
JTW = 128  # j-tile width
NJT = N // JTW  # 32

_BF16 = ml_dtypes.bfloat16

_cached_nc = None


def _build_nc():
    import concourse.bacc as bacc
    import concourse.tile as tile
    import concourse.mybir as mybir

    f32 = mybir.dt.float32
    bf16 = mybir.dt.bfloat16
    Exp = mybir.ActivationFunctionType.Exp
    mult = mybir.AluOpType.mult

    nc = bacc.Bacc("TRN2", target_bir_lowering=False, debug=False)

    xt_d = nc.dram_tensor("xt", [C, N], bf16, kind="ExternalInput").ap()
    wqkv_d = nc.dram_tensor("wqkv", [C, 3 * 128], bf16, kind="ExternalInput").ap()
    wp_d = nc.dram_tensor("wp", [128, C], bf16, kind="ExternalInput").ap()
    bqkv_d = nc.dram_tensor("bqkv", [128, 3], f32, kind="ExternalInput").ap()
    ident_d = nc.dram_tensor("ident", [128, 128], bf16, kind="ExternalInput").ap()
    out_d = nc.dram_tensor("out", [C, N], f32, kind="ExternalOutput").ap()

    with tile.TileContext(nc) as tc:
        with (
            tc.tile_pool(name="ps", space="PSUM", bufs=2) as ps,
            tc.tile_pool(name="sp", bufs=2) as sp,
            tc.tile_pool(name="pe", bufs=1) as pe,
        ):
            # --- persistent SBUF tensors ---
            # x^T loaded in column chunks so QKV chunk 0 can start after
            # the first 4 small DMAs instead of after the full 4MB load.
            xt = [
                pe.tile([128, N], bf16, tag=f"xt{i}", name=f"xt{i}") for i in range(4)
            ]
            for i in range(NIC):
                for ct in range(4):
                    nc.sync.dma_start(
                        out=xt[ct][:, i * ICW : (i + 1) * ICW],
                        in_=xt_d[ct * 128 : (ct + 1) * 128, i * ICW : (i + 1) * ICW],
                    )
            wqkv = pe.tile([128, 4, 3 * 128], bf16, tag="wqkv", name="wqkv")
            nc.sync.dma_start(
                out=wqkv[:], in_=wqkv_d.rearrange("(ct p) m -> p ct m", p=128)
            )
            wp = pe.tile([128, C], bf16, tag="wp", name="wp")
            nc.sync.dma_start(out=wp[:], in_=wp_d[:, :])
            bqkv = pe.tile([128, 3], f32, tag="bqkv", name="bqkv")
            nc.sync.dma_start(out=bqkv[:], in_=bqkv_d[:, :])
            ident = pe.tile([128, 128], bf16, tag="ident", name="ident")
            nc.sync.dma_start(out=ident[:], in_=ident_d[:, :])

            qT = pe.tile([128, N], bf16, tag="qT", name="qT")
            kT = pe.tile([128, N], bf16, tag="kT", name="kT")
            vT = pe.tile([128, N], bf16, tag="vT", name="vT")
            # v in natural layout + ones column: [n-part, jt, head, 64+1]
            vno = pe.tile([128, NJT, HPC, Dh + 1], bf16, tag="vno", name="vno")
            onorm = pe.tile([128, N], bf16, tag="onorm", name="onorm")

            nc.vector.memset(vno[:, :, :, Dh : Dh + 1], 1.0)

            # --- QKV projection + v transpose ---
            dsts = [qT, kT, vT]
            for i in range(NIC):
                isl = slice(i * ICW, (i + 1) * ICW)
                for t in range(3):
                    pqkv = ps.tile([128, ICW], f32, tag="small", bufs=2, name="pqkv")
                    for ct in range(4):
                        nc.tensor.matmul(
                            pqkv[:],
                            lhsT=wqkv[:, ct, t * 128 : (t + 1) * 128],
                            rhs=xt[ct][:, isl],
                            start=(ct == 0),
                            stop=(ct == 3),
                        )
                    nc.vector.tensor_scalar_add(
                        out=dsts[t][:, isl], in0=pqkv[:], scalar1=bqkv[:, t : t + 1]
                    )
                # transpose the v chunks that are now complete: jt in [4i, 4i+4)
                for jt in range(4 * i, 4 * i + 4):
                    pst = ps.tile([128, 128], bf16, tag="small", bufs=2, name="pst")
                    nc.tensor.transpose(
                        pst[:], vT[:, jt * JTW : (jt + 1) * JTW], ident[:]
                    )
                    nc.vector.tensor_copy(
                        out=vno[:, jt, :, 0:Dh],
                        in_=pst[:].rearrange("p (h d) -> p h d", h=HPC),
                    )

            # --- attention (software-pipelined) ---
            # PE stream per unit u=(ic,jt):  scores(u+1) THEN attnV(u), so
            # the ACT exp stream runs back-to-back: exp(u+1) depends only on
            # scores(u+1), which PE computes while ACT is busy with exp(u).
            units = [(ic, jt) for ic in range(NIC) for jt in range(NJT)]
            accs = {}

            def emit_scores(u):
                ic, jt = u
                sc = ps.tile([128, 2 * ICW], f32, tag="sc", bufs=2, name="sc")
                for h in range(HPC):
                    hsl = slice(h * Dh, (h + 1) * Dh)
                    nc.tensor.matmul(
                        sc[:, h * ICW : (h + 1) * ICW],
                        lhsT=kT[hsl, jt * JTW : (jt + 1) * JTW],
                        rhs=qT[hsl, ic * ICW : (ic + 1) * ICW],
                        start=True,
                        stop=True,
                    )
                return sc

            sc_cur = emit_scores(units[0])
            for idx, (ic, jt) in enumerate(units):
                isl = slice(ic * ICW, (ic + 1) * ICW)
                # ACT: exp of current unit's scores
                p = sp.tile([128, 2 * ICW], bf16, tag="p", bufs=3, name="p")
                nc.scalar.activation(p[:], sc_cur[:], Exp, scale=SCALE)
                # PE: next unit's scores (keeps ACT fed while attnV waits)
                if idx + 1 < len(units):
                    sc_cur = emit_scores(units[idx + 1])
                # PE: accumulate attn @ [v | 1] for current unit
                if jt == 0:
                    accs[ic] = [
                        ps.tile([128, ICW], f32, tag=f"acc{h}", bufs=1, name=f"acc{h}")
                        for h in range(HPC)
                    ]
                for h in range(HPC):
                    nc.tensor.matmul(
                        accs[ic][h][0 : Dh + 1, :],
                        lhsT=vno[:, jt, h, :],
                        rhs=p[:, h * ICW : (h + 1) * ICW],
                        start=(jt == 0),
                        stop=(jt == NJT - 1),
                    )
                if jt != NJT - 1:
                    continue
                # --- end of i-chunk: normalize + projection ---
                # normalize: O^T = O_raw^T / denom  (denom = row 64 of acc)
                for h in range(HPC):
                    rc = sp.tile([1, ICW], f32, tag=f"rc{h}", bufs=2, name="rc")
                    nc.vector.reciprocal(rc[:], accs[ic][h][Dh : Dh + 1, :])
                    rb = sp.tile([Dh, ICW], f32, tag=f"rb{h}", bufs=2, name="rb")
                    nc.gpsimd.partition_broadcast(rb[:], rc[:])
                    nc.vector.tensor_tensor(
                        out=onorm[h * Dh : (h + 1) * Dh, isl],
                        in0=accs[ic][h][0:Dh, :],
                        in1=rb[:],
                        op=mult,
                    )
                for cc in range(4):
                    pp = ps.tile([128, ICW], f32, tag="small", bufs=2, name="pp")
                    nc.tensor.matmul(
                        pp[:],
                        lhsT=wp[:, cc * 128 : (cc + 1) * 128],
                        rhs=onorm[:, isl],
                        start=True,
                        stop=True,
                    )
                    st = sp.tile([128, ICW], f32, tag="st", bufs=2, name="st")
                    nc.vector.tensor_copy(out=st[:], in_=pp[:])
                    nc.sync.dma_start(
                        out=out_d[cc * 128 : (cc + 1) * 128, isl], in_=st[:]
                    )

    nc.compile()
    return nc


def get_nc():
    global _cached_nc
    if _cached_nc is None:
        _cached_nc = _build_nc()
    return _cached_nc


def make_in_maps(x, qkv_w, qkv_b, proj_w):
    """Build the per-core input dicts (host-side sharding + layout prep)."""
    x = np.asarray(x, dtype=np.float32)
    qkv_w = np.asarray(qkv_w, dtype=np.float32)
    qkv_b = np.asarray(qkv_b, dtype=np.float32)
    proj_w = np.asarray(proj_w, dtype=np.float32)

    ident = np.eye(128, dtype=_BF16)
    in_maps = []
    for c in range(NCORES):
        b, j = divmod(c, 4)
        rq = slice(128 * j, 128 * (j + 1))
        rk = slice(512 + 128 * j, 512 + 128 * (j + 1))
        rv = slice(1024 + 128 * j, 1024 + 128 * (j + 1))
        xt = np.ascontiguousarray(x[b].T).astype(_BF16)
        wqkv = np.ascontiguousarray(
            np.concatenate(
                [qkv_w[rq].T, qkv_w[rk].T, qkv_w[rv].T], axis=1
            )
        ).astype(_BF16)
        wp = np.ascontiguousarray(proj_w[:, rq].T).astype(_BF16)
        bqkv = np.ascontiguousarray(
            np.stack([qkv_b[rq], qkv_b[rk], qkv_b[rv]], axis=1)
        ).astype(np.float32)
        in_maps.append(
            {"xt": xt, "wqkv": wqkv, "wp": wp, "bqkv": bqkv, "ident": ident}
        )
    return in_maps


def gather_output(results, proj_b):
    """Sum per-core projection partials per batch, transpose, add bias."""
    proj_b = np.asarray(proj_b, dtype=np.float32)
    out = np.empty((B, N, C), dtype=np.float32)
    for b in range(B):
        acc = np.zeros((C, N), dtype=np.float32)
        for j in range(4):
            acc += np.asarray(results[4 * b + j]["out"], dtype=np.float32)
        out[b] = acc.T + proj_b
    return out


def kernel(x, qkv_w, qkv_b, proj_w, proj_b):
    from concourse.bass_utils import run_bass_kernel_spmd

    nc = get_nc()
    in_maps = make_in_maps(x, qkv_w, qkv_b, proj_w)
    res = run_bass_kernel_spmd(nc, in_maps, list(range(NCORES)))
    return gather_output(res.results, proj_b)


def run_traced(x, qkv_w, qkv_b, proj_w, proj_b, trace_cores=None):
    """Like kernel(), but profiles and returns (out, exec_time_ns, raw result)."""
    from concourse.bass_utils import run_bass_kernel_spmd

    nc = get_nc()
    in_maps = make_in_maps(x, qkv_w, qkv_b, proj_w)
    res = run_bass_kernel_spmd(
        nc, in_maps, list(range(NCORES)), trace=True, trace_cores=trace_cores
    )
    return gather_output(res.results, proj_b), res.exec_time_ns, res
